# revision 3
# baseline (speedup 1.0000x reference)
"""Trainium2 Bass kernel for the 2-layer GRU problem (nn_GRU_43568148251487).

Contract: kernel(**inputs) takes the FULL unsharded inputs (batch 64) and
returns the FULL output [64, 512, 64]. Data-parallel over batch across 8
NeuronCores (8 sequences per core), GRU weights replicated, one SPMD Bass
program, no collectives.

v2 design (vs baseline): bf16 recurrence + PE-contiguous schedule.
  - All bulk tensors t-major: row = (t % 16) * 8 + b. Phase A computes gi0
    (fused input projection) into DRAM as bf16; gi1 chunks and the output
    use one [128, .] DMA per 16 steps instead of 8 strided ones.
  - Per layer-step: 15 accumulating bf16 matmuls (N=512) compute
    gates = h@W_hh.T + gi(+bias) in PSUM; the GRU cell runs on ACT+DVE in
    bf16 (DVE 2x mode): sig_r, sig_z, mn=r*pn, mn2=mn+gi_n, n=tanh(mn2),
    s=h-n, m=z*s, h'=n+m. 4 PE transposes regenerate h^T; GpSimd copies
    them into the t-major hist buffer that doubles as lhsT for gi1 bulk
    matmuls and the output projection.
  - PE program order interleaves layer 0 (step s) and layer 1 (step s-32)
    so each layer's cell chain hides under the other layer's matmul block,
    keeping the tensor engine continuously busy (full clock). Per-step gi
    tiles are prefetched 2 supersteps ahead.
"""
import json
from contextlib import ExitStack

import numpy as np
import ml_dtypes

import concourse.bass as bass
import concourse.tile as tile
from concourse import mybir

f32 = mybir.dt.float32
f32r = mybir.dt.float32r
bf16 = mybir.dt.bfloat16
npbf = ml_dtypes.bfloat16
AO = mybir.AluOpType
AF = mybir.ActivationFunctionType

P = 128
B = 8           # batch per core
NCORES = 8
H = 512
G = 3 * H
KC = 4          # k-chunks of the H=512 contraction
CH = 16         # steps per hist chunk
LAG = 32        # layer-1 lag
T_FULL = 512

# ---------------------------------------------------------------------------
# Workaround for this walrus build: it rejects >1 sync-wait per instruction.
# Split extra waits onto preceding EventSemaphore instructions on the same
# engine (same-sequencer program order preserves semantics). Hooked into
# Bass.to_json_bytes so every compile path sees compliant BIR.
_orig_to_json_bytes = bass.Bass.to_json_bytes


def _split_multiwait(mod):
    ctr = [0]

    def mk_es(engine, wait):
        ctr[0] += 1
        return {
            "debug": 0, "engine": engine, "ins": [],
            "name": f"mswsplit-{ctr[0]}", "opcode": "EventSemaphore",
            "outs": [], "sync_info": {"on_update": [], "on_wait": [wait]},
        }

    for fn in mod.get("functions", []):
        for bb in fn.get("blocks", []):
            insts = bb.get("instructions", [])
            if not any(
                len((i.get("sync_info") or {}).get("on_wait") or []) > 1
                for i in insts
            ):
                continue
            out = []
            for inst in insts:
                si = inst.get("sync_info")
                waits = (si or {}).get("on_wait") or []
                if len(waits) > 1:
                    for w in waits[:-1]:
                        out.append(mk_es(inst["engine"], w))
                    si["on_wait"] = [waits[-1]]
                out.append(inst)
            bb["instructions"] = out
    return mod


def _patched_to_json_bytes(self):
    return json.dumps(_split_multiwait(json.loads(_orig_to_json_bytes(self)))).encode()


bass.Bass.to_json_bytes = _patched_to_json_bytes


# ---------------------------------------------------------------------------
def _host_prep(inputs, core):
    x = np.asarray(inputs["x"], np.float32)[core * B:(core + 1) * B]
    t = np.asarray(inputs["t"], np.float32)[core * B:(core + 1) * B]
    T = x.shape[1]
    g = {k: np.asarray(v, np.float32) for k, v in inputs.items()}

    def kchunked(WT, dt):
        F = WT.shape[1]
        return np.ascontiguousarray(
            WT.reshape(KC, P, F).transpose(1, 0, 2).reshape(P, KC * F)
        ).astype(dt)

    b0 = g["b_ih0"] + g["W_ih0"] @ (g["bx"] + g["bt"])
    b0 = b0.copy()
    b0[:2 * H] += g["b_hh0"][:2 * H]
    b1 = g["b_ih1"].copy()
    b1[:2 * H] += g["b_hh1"][:2 * H]
    # t-major row order: row = t*B + b; rows = [x | t | ones] so the fused
    # input-projection matmul also adds the bias (contraction row 65).
    xT = x.transpose(1, 0, 2).reshape(T * B, 64).T
    tT = t.transpose(1, 0, 2).reshape(T * B, 1).T
    xtT = np.concatenate([xT, tT, np.ones((1, T * B), np.float32)], axis=0)
    giAB = np.concatenate(
        [(g["W_ih0"] @ g["Wx"]).T, (g["W_ih0"] @ g["Wt"]).T, b0[None, :]], axis=0)
    return {
        "xtT": np.ascontiguousarray(xtT).astype(np.float32),
        "giAB": np.ascontiguousarray(giAB).astype(np.float32),
        "whhT0": kchunked(g["W_hh0"].T, npbf), "whhT1": kchunked(g["W_hh1"].T, npbf),
        "wihT1": kchunked(g["W_ih1"].T, npbf),
        "bias1": np.ascontiguousarray(np.broadcast_to(b1, (P, G))).astype(np.float32),
        "bhn0": np.ascontiguousarray(g["b_hh0"][2 * H:][None, :]).astype(npbf),
        "bhn1": np.ascontiguousarray(g["b_hh1"][2 * H:][None, :]).astype(npbf),
        "ones8": np.ones((1, B), npbf),
        "id8": np.eye(B, dtype=npbf),
        "woT": kchunked(g["Wo"].T, npbf),
        "bo_bc": np.ascontiguousarray(np.broadcast_to(g["bo"], (P, 64))).astype(np.float32),
    }


def _build(T):
    assert T % CH == 0
    NCHUNK = T // CH       # 32
    NMT = T * B // P       # 32 m-tiles in phase A (t-major rows)
    NSS = T + LAG

    nc = bass.Bass("TRN2", debug=False, num_devices=NCORES)

    d = {}
    d["xtT"] = nc.dram_tensor("xtT", [66, T * B], f32r, kind="ExternalInput")
    d["giAB"] = nc.dram_tensor("giAB", [66, G], f32r, kind="ExternalInput")
    d["whhT0"] = nc.dram_tensor("whhT0", [P, KC * G], bf16, kind="ExternalInput")
    d["whhT1"] = nc.dram_tensor("whhT1", [P, KC * G], bf16, kind="ExternalInput")
    d["wihT1"] = nc.dram_tensor("wihT1", [P, KC * G], bf16, kind="ExternalInput")
    d["bias1"] = nc.dram_tensor("bias1", [P, G], f32, kind="ExternalInput")
    d["bhn0"] = nc.dram_tensor("bhn0", [1, H], bf16, kind="ExternalInput")
    d["bhn1"] = nc.dram_tensor("bhn1", [1, H], bf16, kind="ExternalInput")
    d["ones8"] = nc.dram_tensor("ones8", [1, B], bf16, kind="ExternalInput")
    d["id8"] = nc.dram_tensor("id8", [B, B], bf16, kind="ExternalInput")
    d["woT"] = nc.dram_tensor("woT", [P, KC * 64], bf16, kind="ExternalInput")
    d["bo_bc"] = nc.dram_tensor("bo_bc", [P, 64], f32, kind="ExternalInput")
    # t-major output: [chunk, (t_within, b), 64]; host reorders to [B, T, 64]
    out_d = nc.dram_tensor("out", [NCHUNK, P, 64], f32, kind="ExternalOutput")

    with tile.TileContext(nc) as tc, ExitStack() as ctx:
        wp = ctx.enter_context(tc.tile_pool(name="wp", bufs=1))
        dramp = ctx.enter_context(tc.tile_pool(name="dramp", bufs=1, space="DRAM"))

        def load(name, shape, dt):
            tl = wp.tile(shape, dt, name=f"w_{name}")
            nc.sync.dma_start(tl[:], d[name].ap())
            return tl

        whhT = [load("whhT0", [P, KC * G], bf16), load("whhT1", [P, KC * G], bf16)]
        wihT1 = load("wihT1", [P, KC * G], bf16)
        bias1 = load("bias1", [P, G], f32)
        bhn = [load("bhn0", [1, H], bf16), load("bhn1", [1, H], bf16)]
        ones8 = load("ones8", [1, B], bf16)
        id8 = load("id8", [B, B], bf16)
        woT = load("woT", [P, KC * 64], bf16)
        bo_bc = load("bo_bc", [P, 64], f32)
        giAB = load("giAB", [66, G], f32r)

        hist_init = wp.tile([P, 2, KC, B], bf16, name="hist_init")
        nc.vector.memset(hist_init[:], 0.0)
        hb_init = [wp.tile([B, H], bf16, name=f"hb_init{l}") for l in range(2)]
        for tl in hb_init:
            nc.vector.memset(tl[:], 0.0)

        # gi in DRAM, t-major rows: [m-tile/chunk, (t%16)*8+b, G]
        gi_d = [
            dramp.tile([NMT, P, G], bf16, name="gi0_d"),
            dramp.tile([NCHUNK, P, G], bf16, name="gi1_d"),
        ]

        xtT_sb = wp.tile([66, T * B], f32r, name="xtT_sb")
        nc.sync.dma_start(xtT_sb[:], d["xtT"].ap())

        # ---------------- Phase B: the recurrence (phase A's fused input
        # projection is emitted into the early supersteps, one m-tile per
        # superstep, so it fills the layer-1 pipeline-fill bubble)
        with tc.tile_pool(name="pg", bufs=4) as pg, \
             tc.tile_pool(name="ph", bufs=2) as ph, \
             tc.tile_pool(name="pt", bufs=2) as pt, \
             tc.tile_pool(name="pb", bufs=2) as pb, \
             tc.tile_pool(name="psG", bufs=1, space="PSUM") as psG, \
             tc.tile_pool(name="psT", bufs=1, space="PSUM") as psT, \
             tc.tile_pool(name="psB", bufs=1, space="PSUM") as psB:

            hb_prev = [hb_init[0], hb_init[1]]
            hT_prev = [[hist_init[:, l, k, :] for k in range(KC)] for l in range(2)]
            hist_cur = None
            hist_gen = {}          # L0-chunk generation -> hist tile
            tile_gen = [-1]
            hb_new = [None, None]
            gi_tiles = [{}, {}]

            def gi_load(l, step):
                tl = pg.tile([B, G], bf16, name=f"gi{l}_t")
                j8 = (step % CH) * B
                nc.sync.dma_start(tl[:], gi_d[l][step // CH, j8:j8 + B, :])
                gi_tiles[l][step] = tl

            def transposes_and_copy(l, step, gen):
                """PE-transpose h(l, step) into phT, GpSimd-copy into hist."""
                nonlocal hist_cur
                if gen > tile_gen[0]:
                    hist_cur = ph.tile([P, 2, KC, CH, B], bf16, name="hist")
                    hist_gen[gen] = hist_cur
                    hist_gen.pop(gen - 2, None)
                    tile_gen[0] = gen
                dst = hist_gen[gen]
                slot = step % CH
                phT = psT.tile([P, 2, KC, B], bf16, name="phT", tag="phT")
                for k in range(KC):
                    nc.tensor.transpose(phT[:, l, k, :],
                                        hb_new[l][:, k * P:(k + 1) * P], id8[:])
                nc.vector.tensor_scalar_mul(
                    dst[:, l, :, slot, :], phT[:, l, :, :], 1.0)
                hb_prev[l] = hb_new[l]
                hT_prev[l] = [dst[:, l, k, slot, :] for k in range(KC)]

            def mm_block(l, step, mid_cb=None):
                """gates(l, step) into PSUM from hT_prev and this step's gi.
                mid_cb (the other layer's transposes) is issued between the
                n and z matmul groups so its hist copy overlaps the tail."""
                w = whhT[l]
                gi_t = gi_tiles[l][step]
                przr = psG.tile([B, 512], f32, name=f"przr{l}")
                przz = psG.tile([B, 512], f32, name=f"przz{l}")
                pn = psG.tile([B, 512], f32, name=f"pn{l}")
                # gi/bias injections first: independent of the (late) hist
                # copy, they keep the tensor engine fed through the handoff.
                nc.tensor.matmul(przr[:], id8[:], gi_t[:, 0:512],
                                 start=True, stop=False)
                nc.tensor.matmul(pn[:], ones8[:], bhn[l][:],
                                 start=True, stop=False)
                nc.tensor.matmul(przz[:], id8[:], gi_t[:, 512:1024],
                                 start=True, stop=False)
                # r gate k-minor (its psum unblocks the cell chain first);
                # n/z gates share each hT chunk back-to-back so the stationary
                # operand is loaded once per chunk instead of once per matmul.
                for k in range(KC):
                    nc.tensor.matmul(przr[:], hT_prev[l][k],
                                     w[:, k * G:k * G + 512],
                                     start=False, stop=(k == KC - 1))
                for k in range(KC):
                    nc.tensor.matmul(pn[:], hT_prev[l][k],
                                     w[:, k * G + 1024:k * G + 1536],
                                     start=False, stop=(k == KC - 1))
                    if k == 1 and mid_cb is not None:
                        mid_cb()
                    nc.tensor.matmul(przz[:], hT_prev[l][k],
                                     w[:, k * G + 512:k * G + 1024],
                                     start=False, stop=(k == KC - 1))
                return przr, przz, pn

            def cell(l, step, przr, przz, pn):
                """GRU cell on ACT+DVE in bf16; returns new h [B,H] bf16."""
                gi_t = gi_tiles[l].pop(step)
                arz = pt.tile([B, 1024], bf16, name=f"arz{l}")
                nc.scalar.activation(arz[:, 0:512], przr[:], AF.Sigmoid)
                nc.scalar.activation(arz[:, 512:1024], przz[:], AF.Sigmoid)
                mn = pt.tile([B, H], bf16, name=f"mn{l}")
                nc.vector.tensor_tensor(mn[:], arz[:, 0:512], pn[:], AO.mult)
                nc.vector.tensor_tensor(mn[:], mn[:], gi_t[:, 1024:1536], AO.add)
                n = pt.tile([B, H], bf16, name=f"n{l}")
                nc.scalar.activation(n[:], mn[:], AF.Tanh)
                # s/m/h tail split 448(DVE) | 64(GpSimd) to unload DVE a bit
                s_ = pt.tile([B, H], bf16, name=f"s{l}")
                m_ = pt.tile([B, H], bf16, name=f"m{l}")
                hb = pt.tile([B, H], bf16, name=f"hb{l}")
                for eng, c0, c1 in ((nc.vector, 0, 448), (nc.gpsimd, 448, 512)):
                    eng.tensor_tensor(s_[:, c0:c1], hb_prev[l][:, c0:c1],
                                      n[:, c0:c1], AO.subtract)
                    eng.tensor_tensor(m_[:, c0:c1], arz[:, 512 + c0:512 + c1],
                                      s_[:, c0:c1], AO.mult)
                    eng.tensor_tensor(hb[:, c0:c1], n[:, c0:c1],
                                      m_[:, c0:c1], AO.add)
                return hb

            def emit_phase_a(mt):
                """gi0 m-tile mt: one K=66 matmul per gate + copy to bf16."""
                gi_sb = pb.tile([P, G], bf16, name="gi0_sb")
                for gg in range(3):
                    acc = psB.tile([P, 512], f32, name="accA", tag="accB")
                    nc.tensor.matmul(acc[:], xtT_sb[:, mt * P:(mt + 1) * P],
                                     giAB[:, gg * 512:(gg + 1) * 512],
                                     start=True, stop=True)
                    dstc = gi_sb[:, gg * 512:(gg + 1) * 512]
                    if (mt * 3 + gg) % 2 == 0:
                        nc.scalar.copy(dstc, acc[:])
                    else:
                        nc.vector.tensor_scalar_mul(dstc, acc[:], 1.0)
                nc.sync.dma_start(gi_d[0][mt, :, :], gi_sb[:])

            # bootstrap: first two gi0 m-tiles, then gi loads for supersteps 0/1
            emit_phase_a(0)
            emit_phase_a(1)
            gi_load(0, 0)
            gi_load(0, 1)

            for s in range(NSS + 4):
                t1 = s - LAG
                act0 = s < T
                act1 = 0 <= t1 < T
                p1 = 0 <= t1 - 1 < T

                # 0. remaining phase-A m-tiles ride the pipeline-fill bubble
                if s + 2 < NMT:
                    emit_phase_a(s + 2)
                # prefetch gi two supersteps ahead
                if s + 2 < T:
                    gi_load(0, s + 2)
                if 0 <= t1 + 2 < T:
                    gi_load(1, t1 + 2)

                # 1. layer-0 matmul block (step s), layer-1 transposes of
                #    step t1-1 embedded between its n and z matmul groups.
                mid1 = ((lambda: transposes_and_copy(1, t1 - 1, (s - 1) // CH))
                        if p1 else None)
                if act0:
                    przr0, przz0, pn0 = mm_block(0, s, mid1)
                elif p1:
                    mid1()

                # 2. layer-0 cell (ACT/DVE)
                if act0:
                    hb_new[0] = cell(0, s, przr0, przz0, pn0)

                # 3. layer-1 matmul block (step t1), layer-0 transposes of
                #    step s embedded.
                mid0 = ((lambda: transposes_and_copy(0, s, s // CH))
                        if act0 else None)
                if act1:
                    przr1, przz1, pn1 = mm_block(1, t1, mid0)
                elif act0:
                    mid0()

                # 4. chunk work, spread over 4 supersteps so the shared accB
                #    PSUM bank's WAR never stalls the tensor engine: gi1 gate
                #    group j at s%16==j (j=0,1,2; store after 2), output
                #    projection at s%16==3.
                j = s % CH
                cd = s // CH - 1                # layer-0 chunk complete
                if j in (0, 1, 2) and 0 <= cd < NCHUNK:
                    if j == 0:
                        gi1_sb_cur = pb.tile([P, G], bf16, name="gi1_sb")
                    gg = j
                    accb = psB.tile([P, 512], f32, name="accB", tag="accB")
                    for k in range(KC):
                        nc.tensor.matmul(
                            accb[:], hist_gen[cd][:, 0, k, :, :],
                            wihT1[:, k * G + gg * 512:k * G + (gg + 1) * 512],
                            start=(k == 0), stop=(k == KC - 1))
                    nc.vector.tensor_tensor(
                        gi1_sb_cur[:, gg * 512:(gg + 1) * 512], accb[:],
                        bias1[:, gg * 512:(gg + 1) * 512], AO.add)
                    if j == 2:
                        nc.sync.dma_start(gi_d[1][cd, :, :], gi1_sb_cur[:])
                if j == 3 and 0 <= cd - 2 < NCHUNK:
                    co = cd - 2                 # layer-1 chunk complete
                    rel = pb.tile([P, KC, CH, B], bf16, name="relu_sb")
                    nc.vector.tensor_scalar_max(
                        rel[:], hist_gen[cd][:, 1, :, :, :], 0.0)
                    acco_full = psB.tile([P, 512], f32, name="accO", tag="accB")
                    acco = acco_full[:, 0:64]
                    for k in range(KC):
                        nc.tensor.matmul(acco[:, :], rel[:, k, :, :],
                                         woT[:, k * 64:(k + 1) * 64],
                                         start=(k == 0), stop=(k == KC - 1))
                    out_sb = pb.tile([P, 64], f32, name="out_sb")
                    nc.vector.tensor_tensor(out_sb[:], acco[:], bo_bc[:], AO.add)
                    nc.sync.dma_start(out_d.ap()[co, :, :], out_sb[:])

                # 5. layer-1 cell
                if act1:
                    hb_new[1] = cell(1, t1, przr1, przz1, pn1)
    return nc


_NC_CACHE = {}


def _get_nc(T):
    if T not in _NC_CACHE:
        _NC_CACHE[T] = _build(T)
    return _NC_CACHE[T]


def kernel(**inputs):
    from concourse.bass_utils import run_bass_kernel_spmd

    T = np.asarray(inputs["x"]).shape[1]
    nc = _get_nc(T)
    in_maps = [_host_prep(inputs, c) for c in range(NCORES)]
    res = run_bass_kernel_spmd(nc, in_maps, core_ids=list(range(NCORES)))
    outs = []
    for c in range(NCORES):
        o = np.asarray(res.results[c]["out"])      # [NCHUNK, (CH B), 64]
        o = o.reshape(T // CH, CH, B, 64).transpose(2, 0, 1, 3)
        outs.append(o.reshape(B, T, 64))
    return np.concatenate(outs, axis=0).astype(np.float32)
